# revision 27
# baseline (speedup 1.0000x reference)
"""MHA SPMD kernel v6 for TRN2 (8 cores, head-parallel, fine-grain pipeline).

v6 over v5:
- cross-block software pipeline inside each batch: AV matmuls of block
  i-1 are interleaved (at 2-key-chunk granularity) with the score
  matmuls of block i, so the tensor engine never drains waiting for the
  last exp of a block.  Keeps PE continuously busy -> stays at the high
  p-state clock (0.42 ns/col) instead of dropping to mid (0.83).
- proj filler units reordered q,k-first / v-late so the next batch's
  kt/qt are complete before its first score block.

Output row mapping (per core c):
  y[b*SPC + il*64 + r, :] = out[b, il*IB + c*64 + r, :].
"""

from dataclasses import dataclass

import numpy as np

import concourse.bass as bass
import concourse.bacc as bacc
import concourse.mybir as mybir
import concourse.tile as tile
from concourse.masks import make_identity

F16 = mybir.dt.float16
F32 = mybir.dt.float32
NP_F16 = np.float16


@dataclass
class Cfg:
    B: int = 4
    S: int = 2048
    H: int = 1024
    nh: int = 16
    ncores: int = 8
    IB: int = 512
    EJ: int = 2

    @property
    def dk(self):
        return self.H // self.nh

    @property
    def R(self):
        return self.B * self.S

    @property
    def SPC(self):
        return self.S // self.ncores

    @property
    def KC(self):
        return self.H // 128

    @property
    def JC(self):
        return self.S // 128

    @property
    def NJ(self):
        return self.R // 128


def build_nc(cfg: Cfg, loop_n: int = 0, fake_a2a: bool = False, phases=('proj', 'attn', 'a2a', 'out'), attn_parts=('sc', 'exp', 'av', 'norm'), jcv=None) -> bass.Bass:
    assert cfg.dk == 64
    B, S, H, R, IB, EJ = cfg.B, cfg.S, cfg.H, cfg.R, cfg.IB, cfg.EJ
    KC, JC, NJ, SPC = cfg.KC, cfg.JC, cfg.NJ, cfg.SPC
    NC = cfg.ncores
    assert S % IB == 0 and SPC % 128 == 0 and JC % EJ == 0
    # jcv[b] = number of leading 128-key chunks containing any valid
    # (unmasked) key for batch b, after the host permuted keys valid-first.
    # Chunks >= jcv[b] are never touched; masked keys inside chunk jcv[b]-1
    # contribute 0 via the mask scaling of v and the denominator column.
    if jcv is None:
        jcv = (JC,) * B
    assert len(jcv) == B and all(1 <= j <= JC for j in jcv)
    nch = list(jcv)
    nkv = [-(-n // (IB // 128)) for n in nch]  # k/v proj units per batch

    nc = bacc.Bacc("TRN2")

    xt = nc.declare_dram_parameter("xt", [H, R], F16, isOutput=False)
    wq = nc.declare_dram_parameter("wq_t", [H, 128], F16, isOutput=False)
    wk = nc.declare_dram_parameter("wk_t", [H, 128], F16, isOutput=False)
    wv = nc.declare_dram_parameter("wv_t", [H, 128], F16, isOutput=False)
    wo = nc.declare_dram_parameter("wo_t", [H, H], F16, isOutput=False)
    mv32 = nc.declare_dram_parameter("mask32", [128, NJ], F32, isOutput=False)
    y = nc.declare_dram_parameter("y", [B * SPC, H], F32, isOutput=True)

    NIL = S // IB                      # il blocks per batch
    CW = IB // NC                      # columns per A2A chunk shard (64)
    cc_in = [
        [nc.dram_tensor(f"cc_in{b}_{il}", [NC * 128, CW], F16) for il in range(NIL)]
        for b in range(B)
    ]
    cc_out = [
        [nc.dram_tensor(f"cc_out{b}_{il}", [NC * 128, CW], F16) for il in range(NIL)]
        for b in range(B)
    ]

    xt_r = xt[:].rearrange("(kc p) i -> p kc i", p=128)
    wq_r = wq[:].rearrange("(kc p) m -> p kc m", p=128)
    wk_r = wk[:].rearrange("(kc p) m -> p kc m", p=128)
    wv_r = wv[:].rearrange("(kc p) m -> p kc m", p=128)
    wo_r = wo[:].rearrange("(kc p) n -> p kc n", p=128)

    with tile.TileContext(nc) as tc:
        with tc.tile_pool(name="persist", bufs=1) as persist:
            wq_sb = persist.tile([128, KC, 128], F16)
            wk_sb = persist.tile([128, KC, 128], F16)
            wv_sb = persist.tile([128, KC, 128], F16)
            wo_sb = persist.tile([128, KC, H], F16)
            mv_sb = persist.tile([128, NJ], F32)
            nc.scalar.dma_start(out=wq_sb[:], in_=wq_r)
            nc.scalar.dma_start(out=wk_sb[:], in_=wk_r)
            nc.scalar.dma_start(out=wv_sb[:], in_=wv_r)
            nc.scalar.dma_start(out=mv_sb[:], in_=mv32[:])

            qt_sb = [persist.tile([128, S], F16, name=f"qt{b}") for b in range(B)]
            kt_sb = [persist.tile([128, S], F16, name=f"kt{b}") for b in range(B)]
            v_sb = [
                persist.tile([128, JC, 130], F16, name=f"v{b}") for b in range(B)
            ]
            a_sb = [
                [persist.tile([64, S], F16, name=f"a{b}_{h}") for h in range(2)]
                for b in range(B)
            ]
            ones65 = persist.tile([65, 64], F16)
            nc.vector.memset(ones65[64:65, :], 1.0)
            ident = persist.tile([128, 128], F16)
            make_identity(nc, ident)
            for b in range(B):
                msl = bass.ds(b * JC, JC)
                nc.vector.tensor_copy(
                    v_sb[b][:, :, 64:65],
                    mv_sb[:, msl].rearrange("p (n o) -> p n o", o=1),
                )
                nc.vector.tensor_copy(
                    v_sb[b][:, :, 129:130],
                    mv_sb[:, msl].rearrange("p (n o) -> p n o", o=1),
                )

            with (
                tc.tile_pool(name="xtp", bufs=3) as xtp,
                tc.tile_pool(name="ep", bufs=2) as ep,
                tc.tile_pool(name="rp", bufs=4) as rp,
                tc.tile_pool(name="agp", bufs=2) as agp,
                tc.tile_pool(name="ysb", bufs=2) as ysb,
                tc.tile_pool(name="pmm", bufs=2, space="PSUM") as pmm,
                tc.tile_pool(name="ps", bufs=2, space="PSUM") as ps,
                tc.tile_pool(name="po", bufs=2, space="PSUM") as po,
            ):

                def proj_units(b):
                    """Filler units emitting QKV projection for batch b."""
                    units = []
                    for ibl in range(S // IB):
                        isl = bass.ts(ibl, IB)
                        gsl = bass.ds(b * S + ibl * IB, IB)
                        has_kv = ibl < nkv[b]
                        xt_holder = {}

                        def q_unit(b=b, isl=isl, gsl=gsl, xh=xt_holder):
                            xt_t = xtp.tile(
                                [128, KC, IB], F16, tag="xt", name="xt_t"
                            )
                            hk = KC // 2
                            nc.sync.dma_start(
                                out=xt_t[:, 0:hk], in_=xt_r[:, 0:hk, gsl]
                            )
                            nc.sync.dma_start(
                                out=xt_t[:, hk:KC], in_=xt_r[:, hk:KC, gsl]
                            )
                            xh["t"] = xt_t
                            qp = pmm.tile([128, IB], F32, tag="mm", name="qp")
                            for kc in range(KC):
                                nc.tensor.matmul(
                                    qp[:], wq_sb[:, kc], xt_t[:, kc],
                                    start=(kc == 0), stop=(kc == KC - 1),
                                )
                            nc.vector.tensor_scalar_mul(
                                qt_sb[b][:, isl], qp[:], 0.125
                            )

                        def k_unit(b=b, ibl=ibl, xh=xt_holder):
                            ncols = min(IB, nch[b] * 128 - ibl * IB)
                            kp = pmm.tile([128, IB], F32, tag="mm", name="kp")
                            for kc in range(KC):
                                nc.tensor.matmul(
                                    kp[:, 0:ncols], wk_sb[:, kc],
                                    xh["t"][:, kc, 0:ncols],
                                    start=(kc == 0), stop=(kc == KC - 1),
                                )
                            nc.vector.tensor_copy(
                                kt_sb[b][:, bass.ds(ibl * IB, ncols)],
                                kp[:, 0:ncols],
                            )

                        def v_unit(b=b, ibl=ibl, xh=xt_holder):
                            ncols = min(IB, nch[b] * 128 - ibl * IB)
                            vtp = pmm.tile([128, IB], F32, tag="mm", name="vtp")
                            for kc in range(KC):
                                nc.tensor.matmul(
                                    vtp[:, 0:ncols], wv_sb[:, kc],
                                    xh["t"][:, kc, 0:ncols],
                                    start=(kc == 0), stop=(kc == KC - 1),
                                )
                            vt16 = xtp.tile(
                                [128, IB], F16, tag="vt16", name="vt16"
                            )
                            for t in range(IB // 128):
                                ch = ibl * (IB // 128) + t
                                if ch >= nch[b]:
                                    continue
                                nc.vector.tensor_copy(
                                    vt16[:, bass.ts(t, 128)],
                                    vtp[:, bass.ts(t, 128)],
                                )
                                vp = pmm.tile(
                                    [128, 128], F16, tag="mm", name="vp"
                                )
                                nc.tensor.transpose(
                                    vp[:], vt16[:, bass.ts(t, 128)], ident[:]
                                )
                                mch = b * JC + ch
                                nc.vector.tensor_scalar_mul(
                                    v_sb[b][:, ch, 0:64], vp[:, 0:64],
                                    mv_sb[:, mch : mch + 1],
                                )
                                nc.vector.tensor_scalar_mul(
                                    v_sb[b][:, ch, 65:129], vp[:, 64:128],
                                    mv_sb[:, mch : mch + 1],
                                )

                        units.append(
                            (q_unit, k_unit, v_unit)
                            if ibl < nkv[b] else (q_unit,)
                        )
                    # q/k first (next batch's first score block needs the
                    # full kt and its qt chunk), v late (only needed one
                    # period after the batch boundary, chunk-progressive).
                    # xtp has bufs=3, so v(i) must run before q(i+3).
                    out, vq = [], []
                    for ibl, unit in enumerate(units):
                        if ibl >= 3 and vq:
                            out.append(vq.pop(0))
                        out += list(unit[:2])
                        if len(unit) == 3:
                            vq.append(unit[2])
                    out += vq
                    return out

                def out_proj_units(b):
                    units = []
                    for it in range(SPC // 128):
                        holder = {}

                        def u0(b=b, it=it, hd=holder):
                            ag_t = agp.tile(
                                [128, KC, 128], F16, tag="ag", name="ag_t"
                            )
                            for half in range(128 // CW):
                                il = it * (128 // CW) + half
                                cc_r = cc_out[b][il][:].rearrange(
                                    "(kc p) i -> p kc i", p=128
                                )
                                nc.sync.dma_start(
                                    out=ag_t[:, :, bass.ts(half, CW)], in_=cc_r
                                )
                            y_t = ysb.tile([128, H], F32, tag="y", name="y_t")
                            hd["ag"], hd["y"] = ag_t, y_t
                            yp = pmm.tile([128, 512], F32, tag="mm", name="yp")
                            for kc in range(KC):
                                nc.tensor.matmul(
                                    yp[:], ag_t[:, kc], wo_sb[:, kc, 0:512],
                                    start=(kc == 0), stop=(kc == KC - 1),
                                )
                            nc.vector.tensor_copy(y_t[:, 0:512], yp[:])

                        def u1(b=b, it=it, hd=holder):
                            yp = pmm.tile([128, 512], F32, tag="mm", name="yp")
                            for kc in range(KC):
                                nc.tensor.matmul(
                                    yp[:], hd["ag"][:, kc],
                                    wo_sb[:, kc, 512:1024],
                                    start=(kc == 0), stop=(kc == KC - 1),
                                )
                            nc.vector.tensor_copy(hd["y"][:, 512:1024], yp[:])
                            nc.sync.dma_start(
                                out=y[bass.ds(b * SPC + it * 128, 128), :],
                                in_=hd["y"][:],
                            )

                        units += [u0, u1]
                    return units

                def attn_global(fillq):
                    """One software pipeline over all B*8 blocks: period p
                    issues scores(block p) interleaved with AV(block p-1),
                    normalize(block p-2) mid-period — no per-batch drain
                    bubbles.  fillq[b] = filler units consumed during batch
                    b's periods (2 slots/period, drained with 2-slot margin
                    so batch b+1's first scores never wait on them)."""
                    per_b = (S // IB) * 2
                    blocks_g = [
                        (bb, il, h) for bb in range(B)
                        for il in range(S // IB) for h in range(2)
                    ]
                    nblk_g = len(blocks_g)
                    NWb = [-(-nch[bb] // EJ) for bb in range(B)]
                    st = {}
                    fi = {bb: 0 for bb in range(B)}
                    slot = {bb: 0 for bb in range(B)}
                    SLOT_TGT = 2 * per_b - 2

                    def fill_slot(bb):
                        q = fillq.get(bb, [])
                        slot[bb] += 1
                        s = min(slot[bb], SLOT_TGT)
                        tgt = min(
                            len(q), (len(q) * s + SLOT_TGT - 1) // SLOT_TGT
                        )
                        while fi[bb] < tgt:
                            q[fi[bb]]()
                            fi[bb] += 1

                    def sc_step(i, jw):
                        bb, il, h = blocks_g[i]
                        qsl = bass.ts(il, IB)
                        hsl = bass.ds(h * 64, 64)
                        nj = min(EJ, nch[bb] - jw * EJ)
                        if jw == 0:
                            st[i] = {
                                "e": ep.tile([128, JC, IB], F16, tag="e",
                                             name="e_t")
                            }
                        sp = ps.tile([128, EJ, IB], F32, tag="sp", name="sp")
                        for je in range(nj):
                            jc = jw * EJ + je
                            nc.tensor.matmul(
                                sp[:, je],
                                kt_sb[bb][hsl, bass.ts(jc, 128)],
                                qt_sb[bb][hsl, qsl],
                                start=True, stop=True,
                            )
                        if "exp" in attn_parts:
                            nc.scalar.activation(
                                st[i]["e"][:, bass.ds(jw * EJ, nj)],
                                sp[:, 0:nj],
                                mybir.ActivationFunctionType.Exp,
                            )

                    def av_step(i, jw):
                        bb, il, h = blocks_g[i]
                        nj = min(EJ, nch[bb] - jw * EJ)
                        if jw == 0:
                            st[i]["o"] = po.tile([65, IB], F32, tag="oav",
                                                 name="o2")
                        o2 = st[i]["o"]
                        for je in range(nj):
                            jc = jw * EJ + je
                            nc.tensor.matmul(
                                o2[:],
                                v_sb[bb][:, jc, bass.ds(h * 65, 65)],
                                st[i]["e"][:, jc],
                                start=(jc == 0),
                                stop=(jc == nch[bb] - 1),
                            )

                    def norm_a(i):
                        # DVE half: reciprocal of the denominator row,
                        # written directly as f16 (rb matmul moving input).
                        o2 = st[i]["o"]
                        r16 = rp.tile([65, IB], F16, tag="r16", name="r16")
                        with nc.allow_low_precision(
                            reason="softmax denom ~1e3; f16 recip adds "
                            "~5e-4 rel err, well within tolerance"
                        ):
                            nc.vector.reciprocal(r16[64:65, :], o2[64:65, :])
                        st[i]["r"] = r16

                    def norm_b(i):
                        # PE broadcast + DVE scale; issued a half-period
                        # later so PE never waits on the DVE recip chain.
                        bb, il, h = blocks_g[i]
                        qsl = bass.ts(il, IB)
                        o2 = st[i]["o"]
                        rb = pmm.tile([128, IB], F32, tag="mm", name="rb")
                        nc.tensor.matmul(
                            rb[0:64, :], ones65[64:65, :], st[i]["r"][64:65, :],
                            start=True, stop=True,
                        )
                        rb_sb = rp.tile([64, IB], F32, tag="rbs",
                                        name="rb_sb")
                        nc.vector.tensor_copy(rb_sb[:], rb[0:64, :])
                        nc.vector.tensor_mul(
                            a_sb[bb][h][:, qsl], o2[0:64, :], rb_sb[:]
                        )
                        del st[i]
                        if h == 1 and has("a2a"):
                            a2a_chunk(bb, il)

                    do_sc = "sc" in attn_parts
                    do_full = do_sc and "av" in attn_parts \
                        and "norm" in attn_parts and "exp" in attn_parts
                    for p in range(nblk_g + 2):
                        bb_cur = blocks_g[min(p, nblk_g - 1)][0]
                        NW_sc = NWb[blocks_g[p][0]] if p < nblk_g else 0
                        NW_av = (
                            NWb[blocks_g[p - 1][0]]
                            if 1 <= p <= nblk_g else 0
                        )
                        for jw in range(max(NW_sc, NW_av, 3)):
                            if p < nblk_g and do_sc and jw < NW_sc:
                                sc_step(p, jw)
                            if 1 <= p <= nblk_g and do_full and jw < NW_av:
                                av_step(p - 1, jw)
                            if jw == 2 and p >= 2 and do_full:
                                norm_b(p - 2)
                            if jw == 1:
                                fill_slot(bb_cur)
                        if 1 <= p <= nblk_g and do_full:
                            norm_a(p - 1)
                        fill_slot(bb_cur)
                    for bb in range(B):
                        q = fillq.get(bb, [])
                        while fi[bb] < len(q):
                            q[fi[bb]]()
                            fi[bb] += 1

                def a2a_chunk(b, il):
                    # cc_in[b][il][j*128 + h*64 + p, i] =
                    #   a_sb[b][h][p, il*IB + j*CW + i]
                    for h in range(2):
                        dst = cc_in[b][il][:].rearrange(
                            "(j two p) i -> two p j i", j=NC, two=2
                        )[h]
                        src = a_sb[b][h][:, bass.ds(il * IB, IB)].rearrange(
                            "p (j i) -> p j i", j=NC
                        )
                        nc.sync.dma_start(out=dst, in_=src)
                    if fake_a2a:
                        nc.sync.dma_start(
                            out=cc_out[b][il][:], in_=cc_in[b][il][:]
                        )
                    else:
                        nc.gpsimd.collective_compute(
                            "AllToAll",
                            mybir.AluOpType.bypass,
                            replica_groups=[list(range(NC))],
                            ins=[cc_in[b][il][:]],
                            outs=[cc_out[b][il][:]],
                        )

                has = lambda p: p in phases

                def whole_kernel():
                    if not has("attn"):
                        # phase-isolation mode: just run requested phases flat
                        if has("proj"):
                            for b in range(B):
                                for u in proj_units(b):
                                    u()
                        if has("out"):
                            nc.sync.dma_start(out=wo_sb[:], in_=wo_r)
                            for b in range(B):
                                for u in out_proj_units(b):
                                    u()
                        return
                    tail0 = []
                    if has("proj"):
                        u0 = proj_units(0)
                        # keep the last two (v) units as early attn(0)
                        # fillers -> shorter serial prologue
                        u0, tail0 = u0[:-2], u0[-2:]
                        for u in u0:
                            u()
                    nc.sync.dma_start(out=wo_sb[:], in_=wo_r)
                    fillq = {}
                    for b in range(B):
                        q = list(tail0) if b == 0 else []
                        if has("proj") and b + 1 < B:
                            q += proj_units(b + 1)
                        elif has("out") and b == B - 1:
                            for pb in range(B - 1):
                                q += out_proj_units(pb)
                        fillq[b] = q
                    attn_global(fillq)
                    if has("out"):
                        for u in out_proj_units(B - 1):
                            u()

                def attn_prereq():
                    # materialize q/k/v once, outside the timing loop
                    for b in range(B):
                        for u in proj_units(b):
                            u()

                if loop_n > 0:
                    if has("attn") and not has("proj"):
                        attn_prereq()
                    with tc.For_i(0, loop_n):
                        whole_kernel()
                else:
                    whole_kernel()

    nc.finalize()
    return nc


# ---------------------------------------------------------------------------


def make_inputs(cfg: Cfg, x, mask, Wq, Wk, Wv, Wo):
    """Host prep: per-batch permute keys/queries valid-first (attention is
    permutation-invariant over keys; queries are un-permuted in
    assemble_output).  Returns (per-core inputs, jcv, perms)."""
    B, S, H, NC = cfg.B, cfg.S, cfg.H, cfg.ncores
    m01 = (np.asarray(mask).reshape(B, S) != 0)
    perms = np.empty((B, S), np.int64)
    jcv = []
    for b in range(B):
        perms[b] = np.argsort(~m01[b], kind="stable")
        sv = int(m01[b].sum())
        jcv.append(min(cfg.JC, max(1, -(-sv // 128))))
    jcv = tuple(jcv)
    xp = np.empty_like(x)
    m01p = np.empty_like(m01)
    for b in range(B):
        xp[b] = x[b, perms[b]]
        m01p[b] = m01[b, perms[b]]
    xt = np.ascontiguousarray(xp.reshape(B * S, H).T.astype(NP_F16))
    wo_t = np.ascontiguousarray(Wo.T.astype(NP_F16))
    mcol = np.ascontiguousarray(
        m01p.astype(np.float32).reshape(cfg.NJ, 128).T
    )
    ins = []
    for c in range(NC):
        blk = slice(c * 128, (c + 1) * 128)
        ins.append(
            {
                "xt": xt,
                "wq_t": np.ascontiguousarray(Wq[blk, :].T.astype(NP_F16)),
                "wk_t": np.ascontiguousarray(Wk[blk, :].T.astype(NP_F16)),
                "wv_t": np.ascontiguousarray(Wv[blk, :].T.astype(NP_F16)),
                "wo_t": wo_t,
                "mask32": mcol,
            }
        )
    return ins, jcv, perms


def assemble_output(cfg: Cfg, per_core_y, bo, perms=None):
    B, S, H, SPC, IB = cfg.B, cfg.S, cfg.H, cfg.SPC, cfg.IB
    NC = cfg.ncores
    CW = IB // NC
    NIL = S // IB
    out = np.empty((B, S, H), np.float32)
    for c, yc in enumerate(per_core_y):
        yc = np.asarray(yc).reshape(B, NIL, CW, H)
        for b in range(B):
            for il in range(NIL):
                out[b, il * IB + c * CW : il * IB + (c + 1) * CW] = yc[b, il]
    if perms is not None:
        for b in range(B):
            out[b, perms[b]] = out[b].copy()
    out += bo.astype(np.float32)[None, None, :]
    return out


def reference_np(cfg: Cfg, x, mask, Wq, Wk, Wv, Wo, bo):
    B, S, H, nh, dk = cfg.B, cfg.S, cfg.H, cfg.nh, cfg.dk
    xf = x.reshape(B * S, H).astype(np.float64)
    out = np.zeros((B, S, H), np.float64)
    for b in range(B):
        xb = xf[b * S : (b + 1) * S]
        mrow = mask.reshape(B, S)[b]
        A = np.zeros((S, H), np.float64)
        for h in range(nh):
            q = xb @ Wq[h * dk : (h + 1) * dk].T.astype(np.float64) / np.sqrt(dk)
            k = xb @ Wk[h * dk : (h + 1) * dk].T.astype(np.float64)
            v = xb @ Wv[h * dk : (h + 1) * dk].T.astype(np.float64)
            sc = q @ k.T
            sc = np.where(mrow[None, :] == 0, -1e9, sc)
            e = np.exp(sc - sc.max(-1, keepdims=True))
            p = e / e.sum(-1, keepdims=True)
            A[:, h * dk : (h + 1) * dk] = p @ v
        out[b] = A @ Wo.T.astype(np.float64)
    return (out + bo[None, None, :]).astype(np.float32)


# ---------------------------------------------------------------------------
# harness entry point: full inputs in, full output out

_CACHED = {}


def kernel(x, mask, Wq, Wk, Wv, Wo, bo):
    """Multi-head attention on 8 TRN2 NeuronCores (head-parallel TP).

    Sharding: 2 heads per core (Wq/Wk/Wv split by head rows = column-wise
    per the torch convention); scores/softmax/AV computed in keys-on-
    partition layout with the mask folded into the V-augmented matmul
    (ones column -> softmax denominator); 16 small AllToAll collectives
    redistribute the head-sharded attention output to seq-sharded form,
    fired as each 512-query block completes so they overlap compute; each
    core then computes its 1/8 of output rows against full Wo.  The host
    only transposes/casts inputs, concatenates outputs and adds the bias.
    """
    from concourse.bass_utils import run_bass_kernel_spmd

    x = np.ascontiguousarray(np.asarray(x, dtype=np.float32))
    mask = np.asarray(mask)
    Wq = np.asarray(Wq, dtype=np.float32)
    Wk = np.asarray(Wk, dtype=np.float32)
    Wv = np.asarray(Wv, dtype=np.float32)
    Wo = np.asarray(Wo, dtype=np.float32)
    bo = np.asarray(bo, dtype=np.float32)

    cfg = Cfg(B=x.shape[0], S=x.shape[1], H=x.shape[2])
    ins, jcv, perms = make_inputs(cfg, x, mask, Wq, Wk, Wv, Wo)
    if _CACHED.get("jcv") != jcv:
        _CACHED["nc"] = build_nc(cfg, jcv=jcv)
        _CACHED["jcv"] = jcv
    nc = _CACHED["nc"]

    res = run_bass_kernel_spmd(nc, ins, list(range(cfg.ncores)))
    ys = [res.results[c]["y"] for c in range(cfg.ncores)]
    return assemble_output(cfg, ys, bo, perms).astype(np.float32)

